# revision 42
# baseline (speedup 1.0000x reference)
"""Pairwise Euclidean distance matrix on 8 Trainium2 NeuronCores.

Problem: mapping [8192, 512] f32 -> out[i,j] = ||mapping_i - mapping_j||_2,
shape [8192, 8192] f32.

Strategy: symmetry-aware staircase sharding + fp8 DoubleRow matmuls.

The output is symmetric, so only ~half needs computing on device. Rows are
split into 16 blocks of 512; row-block R computes column blocks
C = R..R+8 (mod 16) - a 4608-wide rotated band. Every unordered block pair
{R, C} is covered (distance k=|C-R| mod 16 <= 8 directly, k > 8 via the
transposed partner), so the host mirrors the missing blocks. Core c owns
row-blocks {2c, 2c+1} (1024 rows, two 512-row strips); the two strips'
bands overlap so their union [1024c, 1024c+5120) mod 8192 is loaded once.
Work per core: 72 [128,512] output tiles = 56% of the dense row slab.

Math per tile: d^2 = sq_m + sq_n - 2 a_m.a_n on fp8(e4m3)-rounded points.
  - Gram: TensorE fp8 DoubleRow matmuls (2 contraction rows per partition,
    2x fp16 throughput; 512-dim contraction = 2 matmuls per [128,512] tile,
    216 ns each at full clock). Moving operand is A^T (shared band);
    stationary is -2*A rows.
  - Epilogue, uniform per 4-bank PSUM group: DVE adds an fp16 sq_n
    broadcast row ([128,2048], ~2.3 us - the pipeline governor and the
    PSUM release point), ACT computes sqrt(t + sq_m) with per-partition
    f32 bias, fp16 out, then one store DMA. Keeping every op group-sized
    and DVE-released matters: ACT-released PSUM or merged multi-group ACTs
    stall the PE, whose clock then drops to the 1.2 GHz p-state (427 ns
    matmuls) until ~3 us of continuous work re-ramps it.
  - Band block 8 groups put m-tiles in banks (same columns 4x): sq_n comes
    from a host-tiled row (sqnf), sq_m via four per-bank ACT biases.
  - Warmup: fp8-DoubleRow dummy matmuls ramp the PE clock and its weight
    path while inputs stream in (fp16 warmups leave the first ~13 real
    DoubleRow matmuls at 427 ns).

Host side (cheap, O(N^2) only for unshard/mirror): fp8 rounding of points,
sq in f64, strip gather (mod-rotation), band placement, symmetric mirror of
the uncomputed blocks, diagonal zero. The device computes every distance
at least once.
"""

import numpy as np
import ml_dtypes
import bass_rust
import concourse.bass as bass
import concourse.mybir as mybir
from concourse.tile import TileContext
from concourse.bass_utils import run_bass_kernel_spmd


N = 8192
D = 512
NCORES = 8
NB = 512                  # block size (rows/cols)
NBLK = N // NB            # 16 row/col blocks
BAND = 9                  # col blocks computed per row block
BANDW = BAND * NB         # 4608
UNIW = BANDW + NB         # 5120: union of the two strips' bands
ROWS = 1024               # rows per core (2 strips)
F32 = mybir.dt.float32
F16 = mybir.dt.float16
F8 = mybir.dt.float8e4
NP8 = ml_dtypes.float8_e4m3
DBL = mybir.MatmulPerfMode.DoubleRow
ADD = mybir.AluOpType.add
SQRT = mybir.ActivationFunctionType.Sqrt


def _split_excess_waits(nc, limit=1):
    """Walrus in this container rejects instructions with >1 sem-wait; hoist
    excess waits onto same-engine NoOps (stream order preserves blocking)."""
    for fn in nc.m.functions:
        for blk in fn.blocks:
            newlist = []
            changed = False
            for ins in blk.instructions:
                si = ins.sync_info
                if si is not None and si.on_wait and len(si.on_wait) > limit:
                    waits = list(si.on_wait)
                    excess, keep = waits[:-limit], waits[-limit:]
                    for i, w in enumerate(excess):
                        nop = bass_rust.InstNoOp(
                            name=f"{ins.name}-wsplit{i}", ins=[], outs=[]
                        )
                        nop.engine = ins.engine
                        nop.sync_info = mybir.SyncInfo(on_wait=[w], on_update=[])
                        newlist.append(nop)
                    si.on_wait = keep
                    ins.sync_info = si
                    changed = True
                newlist.append(ins)
            if changed:
                blk.instructions = newlist


def _build():
    nc = bass.Bass()
    # A^T union band, 2 contraction double-chunks: [k, i, n] = AT[256c+128i+k]
    at0_d = nc.dram_tensor("at0", [128, 2, UNIW], F8, kind="ExternalInput")
    at1_d = nc.dram_tensor("at1", [128, 2, UNIW], F8, kind="ExternalInput")
    # -2*A rows (stationary): [k, i, m] over the core's 1024 rows
    lhs0_d = nc.dram_tensor("lhs0", [128, 2, ROWS], F8, kind="ExternalInput")
    lhs1_d = nc.dram_tensor("lhs1", [128, 2, ROWS], F8, kind="ExternalInput")
    sqn_d = nc.dram_tensor("sqn", [1, UNIW], F16, kind="ExternalInput")
    sqm_d = nc.dram_tensor("sqm", [128, 8], F32, kind="ExternalInput")
    # block-8 sq_n row, repeated 4x (one copy per m-tile bank), per strip
    sqnf_d = nc.dram_tensor("sqnf", [1, 4096], F16, kind="ExternalInput")
    out_d = nc.dram_tensor("out", [ROWS, BANDW], F16, kind="ExternalOutput")

    with TileContext(nc) as tc:
        with (
            tc.tile_pool(name="const", bufs=1) as cpool,
            tc.tile_pool(name="ps", bufs=2, space="PSUM") as pspool,
            tc.tile_pool(name="t", bufs=4) as tpool,
            tc.tile_pool(name="u", bufs=6) as upool,
        ):
            # Allocate big tiles first: every slot below is a multiple of
            # 2048 B/partition, keeping all bases 64B-aligned (a 32B-aligned
            # DVE operand measured ~20% slower). 32B-slot tiles come last.
            atb = [cpool.tile([128, 2, UNIW], F8, tag=f"atb{ch}", name=f"atb{ch}")
                   for ch in range(2)]
            sqn = cpool.tile([128, UNIW], F16)
            lhs = [cpool.tile([128, 2, ROWS], F8, tag=f"lhs{ch}", name=f"lhs{ch}")
                   for ch in range(2)]
            sqnf = cpool.tile([128, 4096], F16)
            warm_in = cpool.tile([128, 2, 512], F8)
            warm_act = cpool.tile([128, 16], F32)
            sqm = cpool.tile([128, 8], F32)

            nc.vector.memset(warm_in[:], 1.0)
            nc.vector.memset(warm_act[:], 1.0)

            # PE clock-gate warmup (HAM ramp) with the same fp8 DoubleRow
            # shape the real matmuls use: short N=64 ops ramp the clock
            # cheaply, N=512 finishers reach full p-state in-mode.
            warm_ps = pspool.tile([128, 2048], F32, tag="ps")
            for _ in range(30):
                nc.tensor.matmul(
                    warm_ps[:, 0:64], warm_in[:, 0:2, 0:128],
                    warm_in[:, 0:2, 0:64],
                    start=True, stop=True, perf_mode=DBL,
                )
            for _ in range(6):
                nc.tensor.matmul(
                    warm_ps[:, 0:512], warm_in[:, 0:2, 0:128],
                    warm_in[:, 0:2, 0:512],
                    start=True, stop=True, perf_mode=DBL,
                )
            # ACT Sqrt table preload
            nc.scalar.activation(warm_act[:], warm_act[:], SQRT, bias=0.0)

            # Input DMAs on the sync queue, ordered by when compute needs
            # them; outputs share the queue (a gpsimd SWDGE output queue
            # costs a 3.3 us dge-drain at teardown). atb slices move both
            # k-subtile rows per DMA.
            nc.sync.dma_start(sqm[:], sqm_d[:])
            for ch, ld, ad in ((0, lhs0_d, at0_d), (1, lhs1_d, at1_d)):
                nc.sync.dma_start(lhs[ch][:], ld[:])
                nc.sync.dma_start(atb[ch][:, :, 0:2048], ad[:, :, 0:2048])
            nc.sync.dma_start(
                sqn[:, 0:2048], sqn_d[0:1, 0:2048].partition_broadcast(128)
            )
            for ch, ad in enumerate((at0_d, at1_d)):
                nc.sync.dma_start(atb[ch][:, :, 2048:4096], ad[:, :, 2048:4096])
            nc.sync.dma_start(
                sqn[:, 2048:UNIW],
                sqn_d[0:1, 2048:UNIW].partition_broadcast(128),
            )
            for ch, ad in enumerate((at0_d, at1_d)):
                nc.sync.dma_start(atb[ch][:, :, 4096:UNIW], ad[:, :, 4096:UNIW])
            nc.sync.dma_start(
                sqnf[:], sqnf_d[0:1, :].partition_broadcast(128)
            )

            # Uniform per-group pipeline: mms -> DVE add -> ACT sqrt -> DMA,
            # every op <= one group wide so no engine builds a backlog.
            # Group order per strip keeps early groups on early atb columns;
            # folds sit mid-strip where the ACT queue is shallow.
            # Uniform per-group pipeline: mms -> DVE add -> ACT sqrt -> DMA,
            # every op <= one group wide so no engine builds a backlog, and
            # DVE releases every PSUM group (ACT-released groups measurably
            # stall the PE and drop its p-state). Group order per strip keeps
            # early groups on early atb columns.
            for s in range(2):      # strip = row half
                base = NB * s       # band offset in union cols
                if s == 0:
                    order = [(0, 0), (1, 0), (0, 1), (2, 0), (1, 1), (3, 0),
                             (2, 1), (3, 1), "fold"]
                else:
                    order = [(0, 0), (0, 1), (1, 0), (1, 1), (2, 0),
                             (2, 1), (3, 0), "fold", (3, 1)]
                for item in order:
                    if item == "fold":
                        # band block 8: bank b holds m-tile b's block-8 cols.
                        # Same DVE-release pipeline as every other group; the
                        # sq_n row comes 4x-repeated via sqnf, sq_m via 4
                        # per-bank ACT biases.
                        c0 = base + 4096
                        ps = pspool.tile([128, 2048], F32, tag="ps")
                        for m in range(4):
                            m0 = NB * s + 128 * m
                            for ch in range(2):
                                nc.tensor.matmul(
                                    ps[:, 512 * m:512 * (m + 1)],
                                    lhs[ch][:, 0:2, m0:m0 + 128],
                                    atb[ch][:, 0:2, c0:c0 + 512],
                                    start=(ch == 0), stop=(ch == 1),
                                    perf_mode=DBL,
                                )
                        tf = tpool.tile([128, 2048], F16, tag="t")
                        nc.vector.tensor_tensor(
                            tf[:], ps[:],
                            sqnf[:, 2048 * s:2048 * (s + 1)], ADD
                        )
                        uf = upool.tile([128, 2048], F16, tag="u")
                        for m in range(4):
                            m0 = NB * s + 128 * m
                            nc.scalar.activation(
                                uf[:, 512 * m:512 * (m + 1)],
                                tf[:, 512 * m:512 * (m + 1)], SQRT,
                                bias=sqm[:, 4 * s + m:4 * s + m + 1],
                            )
                            nc.sync.dma_start(
                                out_d[m0:m0 + 128, 4096:4608],
                                uf[:, 512 * m:512 * (m + 1)],
                            )
                        continue
                    m, g = item
                    mt = 4 * s + m
                    m0 = NB * s + 128 * m
                    c0 = base + 2048 * g
                    first = (s == 0 and item == (0, 0))
                    last = (s == 1 and item == (3, 1))
                    ps = pspool.tile([128, 2048], F32, tag="ps")
                    # ch-major in the first group: chunk-0 mms start as soon
                    # as chunk 0's columns land, chunk 1 still in flight
                    bch = ([(b, ch) for ch in range(2) for b in range(4)]
                           if first else
                           [(b, ch) for b in range(4) for ch in range(2)])
                    for b, ch in bch:
                        nb0 = c0 + 512 * b
                        nc.tensor.matmul(
                            ps[:, 512 * b:512 * (b + 1)],
                            lhs[ch][:, 0:2, m0:m0 + 128],
                            atb[ch][:, 0:2, nb0:nb0 + 512],
                            start=(ch == 0), stop=(ch == 1),
                            perf_mode=DBL,
                        )
                    t = tpool.tile([128, 2048], F16, tag="t")
                    u = upool.tile([128, 2048], F16, tag="u")
                    halves = ((0, 1024), (1024, 2048)) if last \
                        else ((0, 2048),)
                    for lo, hi in halves:
                        nc.vector.tensor_tensor(
                            t[:, lo:hi], ps[:, lo:hi],
                            sqn[:, c0 + lo:c0 + hi], ADD
                        )
                        nc.scalar.activation(
                            u[:, lo:hi], t[:, lo:hi], SQRT,
                            bias=sqm[:, mt:mt + 1]
                        )
                        nc.sync.dma_start(
                            out_d[m0:m0 + 128, 2048 * g + lo:2048 * g + hi],
                            u[:, lo:hi],
                        )
    _split_excess_waits(nc)
    return nc


def prepare_in_maps(mapping: np.ndarray):
    mapping = np.ascontiguousarray(mapping, dtype=np.float32)
    assert mapping.shape == (N, D)
    a8 = mapping.astype(NP8)
    af = a8.astype(np.float32)
    # exact squared norms of the rounded points
    sq = np.einsum("nd,nd->n", af.astype(np.float64),
                   af.astype(np.float64)).astype(np.float64)
    lhs8 = (-2.0 * af).astype(NP8)           # exact: *2 shifts exponent
    at8 = np.ascontiguousarray(a8.T)         # [D, N]
    lhs8t = np.ascontiguousarray(lhs8.T)     # [D, N]

    in_maps = []
    for c in range(NCORES):
        cols = (1024 * c + np.arange(UNIW)) % N
        atr = np.take(at8, cols, axis=1)     # [512, 5120]
        rows = slice(1024 * c, 1024 * c + ROWS)
        lhsr = lhs8t[:, rows]                # [512, 1024]

        def chunked(x, ch):
            # [256, W] rows 256ch..256ch+256 -> [128, 2, W]
            blk = x[256 * ch:256 * (ch + 1)]
            return np.ascontiguousarray(
                blk.reshape(2, 128, -1).transpose(1, 0, 2)
            )

        sqm = np.ascontiguousarray(
            sq[rows].reshape(8, 128).T.astype(np.float32)
        )                                    # [128, 8][p, mt]
        sqn = sq[cols].astype(np.float16).reshape(1, UNIW)
        # block-8 sq rows, repeated 4x, per strip
        sqnf = np.empty((1, 4096), np.float16)
        for s in range(2):
            blk8 = sq[cols[512 * s + 4096:512 * s + 4608]].astype(np.float16)
            sqnf[0, 2048 * s:2048 * (s + 1)] = np.tile(blk8, 4)
        in_maps.append({
            "at0": chunked(atr, 0), "at1": chunked(atr, 1),
            "lhs0": chunked(lhsr, 0), "lhs1": chunked(lhsr, 1),
            "sqn": sqn, "sqm": sqm, "sqnf": sqnf,
        })
    return in_maps


def assemble(results) -> np.ndarray:
    """Place the 16 computed band strips, mirror the missing blocks."""
    out = np.empty((N, N), dtype=np.float32)
    for c in range(NCORES):
        band = results[c]["out"].astype(np.float32)   # [1024, 4608]
        for s in range(2):
            r0 = 1024 * c + NB * s
            strip = band[NB * s:NB * s + NB]
            c0 = r0 % N
            w1 = min(BANDW, N - c0)
            out[r0:r0 + NB, c0:c0 + w1] = strip[:, :w1]
            if w1 < BANDW:
                out[r0:r0 + NB, 0:BANDW - w1] = strip[:, w1:]
    # mirror blocks with (C-R) mod 16 in 9..15 from their transposed partner
    for k in range(BAND, NBLK):
        for R in range(NBLK):
            C = (R + k) % NBLK
            out[R * NB:(R + 1) * NB, C * NB:(C + 1) * NB] = \
                out[C * NB:(C + 1) * NB, R * NB:(R + 1) * NB].T
    np.fill_diagonal(out, 0.0)
    return out


_NC_CACHE = {}


def kernel(mapping: np.ndarray) -> np.ndarray:
    in_maps = prepare_in_maps(mapping)
    if "nc" not in _NC_CACHE:
        _NC_CACHE["nc"] = _build()
    nc = _NC_CACHE["nc"]
    res = None
    for attempt in range(3):
        try:
            res = run_bass_kernel_spmd(nc, in_maps, core_ids=list(range(NCORES)))
            break
        except Exception:
            # transient device wedge; pause + retry
            if attempt == 2:
                raise
            import time
            time.sleep(20)
    return assemble([res.results[c] for c in range(NCORES)])
